# revision 9
# baseline (speedup 1.0000x reference)
"""CosineSimilarityLoss (histogram binning) Trainium2 kernel.

Full inputs [2048, 4096] f32 x5 -> scalar f32 loss = 1 - mean(cosine_sim).

Strategy: data-parallel over 8 cores (256 rows each). Per core, per
128-row tile: stream y = mz*2000 and intensity through a fused custom
DVE instruction  accum[p] = sum_k I[p,k] * [j <= y[p,k] < j+1]  --
one instruction per histogram bin -- building per-row histograms
PH/TH [128, 2000] exactly (bit-exact trunc semantics vs the reference's
astype(int32)). Dot products and squared norms via tensor_tensor_reduce,
cosine tail math on-chip; final mean on host.
"""

import sys

sys.path.insert(0, "/opt/trn_rl_repo")

from operator import add

import numpy as np

import concourse.bass as bass
from concourse import bacc, mybir
from concourse.bass_utils import run_bass_kernel_spmd
from concourse.tile import TileContext
from concourse.dve_ops import (
    DveOp,
    OPS,
    CUSTOM_DVE_SPECS,
    _SUB_OPCODE_FOR_NAME,
    _CUSTOM_DVE_ROW_BASE,
)
from concourse.dve_spec import Spec, Src0, Src1, C0, C1, Zero, lower
from concourse.dve_uop import DveOpSpec

import os

N_CORES = 8
B, P, T = 2048, 4096, 4096
ROWS_PER_CORE = B // N_CORES  # 256
NUM_BINS = int(os.environ.get("KERNEL_NUM_BINS", "2000"))
STAGE = os.environ.get("KERNEL_STAGE", "full")  # hist | red | full
EPS = 1e-8
F32 = mybir.dt.float32
ALU = mybir.AluOpType


def _register_op(name, spec):
    shas = {}
    for ver in ("v3", "v4"):
        s = DveOpSpec(name=name, opcode=0, uops=lower(spec, ver=ver), rd1_en=True)
        shas[ver] = s.sha(ver)
    op = DveOp(name, spec, subdim=False, uops_sha=shas)
    if name not in _SUB_OPCODE_FOR_NAME:
        OPS.append(op)
        CUSTOM_DVE_SPECS[name] = spec
        _SUB_OPCODE_FOR_NAME[name] = _CUSTOM_DVE_ROW_BASE + len(OPS) - 1
        assert _SUB_OPCODE_FOR_NAME[name] < 0x20
    return op


def _win_ref(in0, in1, s0, s1, imm2):
    b = ((in0 >= s0) & (in0 < s1)).astype(np.float32) * in1
    return b, b.reshape(b.shape[0], -1).sum(axis=-1, keepdims=True).astype(np.float32)


# accum_out[p] = sum_k in1[p,k] * [s0 <= in0[p,k] < s1]
WINMULRED = _register_op(
    "WINMULRED",
    Spec(
        body=(Src0 >= C0) * (Src0 < C1) * Src1,
        accum=add,
        accum_init=Zero,
        reference=_win_ref,
    ),
)

_NC_CACHE = {}


def build_nc():
    if "nc" in _NC_CACHE:
        return _NC_CACHE["nc"]
    nc = bacc.Bacc("TRN2", target_bir_lowering=False, debug=False, num_devices=N_CORES)
    d_pmz = nc.dram_tensor("pmz", [ROWS_PER_CORE, P], F32, kind="ExternalInput")
    d_pint = nc.dram_tensor("pint", [ROWS_PER_CORE, P], F32, kind="ExternalInput")
    d_tmz = nc.dram_tensor("tmz", [ROWS_PER_CORE, T], F32, kind="ExternalInput")
    d_tint = nc.dram_tensor("tint", [ROWS_PER_CORE, T], F32, kind="ExternalInput")
    d_tmask = nc.dram_tensor("tmask", [ROWS_PER_CORE, T], F32, kind="ExternalInput")
    d_cos = nc.dram_tensor("cos", [128, 2], F32, kind="ExternalOutput")

    n_tiles = ROWS_PER_CORE // 128  # 2

    with TileContext(nc) as tc:
        with (
            tc.tile_pool(name="io", bufs=1) as io,
            tc.tile_pool(name="hist", bufs=1) as hist,
            tc.tile_pool(name="red", bufs=1) as red,
        ):
            scr = red.tile([128, P], F32, tag="scr")
            dot = red.tile([128, 2], F32, tag="dot")
            pn2 = red.tile([128, 2], F32, tag="pn2")
            tn2 = red.tile([128, 2], F32, tag="tn2")

            for t in range(n_tiles):
                rs = slice(128 * t, 128 * (t + 1))
                yp = io.tile([128, P], F32, tag="yp")
                ip = io.tile([128, P], F32, tag="ip")
                yt = io.tile([128, T], F32, tag="yt")
                it = io.tile([128, T], F32, tag="it")
                tm = io.tile([128, T], F32, tag="tm")
                nc.sync.dma_start(yp[:], d_pmz[rs, :])
                nc.sync.dma_start(ip[:], d_pint[rs, :])
                nc.sync.dma_start(yt[:], d_tmz[rs, :])
                nc.sync.dma_start(it[:], d_tint[rs, :])
                nc.sync.dma_start(tm[:], d_tmask[rs, :])
                # y = mz * 2000  (single-rounded f32 mul, matches reference)
                nc.vector.tensor_scalar_mul(yp[:], yp[:], 2000.0)
                nc.vector.tensor_scalar_mul(yt[:], yt[:], 2000.0)
                # masked target intensity
                nc.vector.scalar_tensor_tensor(
                    it[:], it[:], 0.0, tm[:], ALU.bypass, ALU.mult
                )

                ph = hist.tile([128, NUM_BINS], F32, tag="ph")
                th = hist.tile([128, NUM_BINS], F32, tag="th")
                for j in range(NUM_BINS):
                    nc.vector._custom_dve(
                        WINMULRED,
                        out=scr[:],
                        accum_out=ph[:, j : j + 1],
                        in0=yp[:],
                        in1=ip[:],
                        s0=float(j),
                        s1=float(j + 1),
                    )
                for j in range(NUM_BINS):
                    nc.vector._custom_dve(
                        WINMULRED,
                        out=scr[:],
                        accum_out=th[:, j : j + 1],
                        in0=yt[:],
                        in1=it[:],
                        s0=float(j),
                        s1=float(j + 1),
                    )
                if STAGE == "hist":
                    continue
                hsc = red.tile([128, NUM_BINS], F32, tag="hsc")
                nc.vector.scalar_tensor_tensor(
                    hsc[:], ph[:], 0.0, th[:], ALU.bypass, ALU.mult,
                    accum_out=dot[:, t : t + 1],
                )
                nc.scalar.activation(
                    hsc[:], ph[:], mybir.ActivationFunctionType.Square,
                    accum_out=pn2[:, t : t + 1],
                )
                nc.scalar.activation(
                    hsc[:], th[:], mybir.ActivationFunctionType.Square,
                    accum_out=tn2[:, t : t + 1],
                )

            if STAGE == "hist":
                cz = red.tile([128, 2], F32, tag="cz")
                nc.vector.tensor_scalar_mul(cz[:], scr[:, 0:2], 0.0)
                nc.sync.dma_start(d_cos[:], cz[:])
            elif STAGE == "red":
                nc.sync.dma_start(d_cos[:], dot[:])
            else:
                _build_tail(nc, red, dot, pn2, tn2, d_cos)
    nc.compile()
    _NC_CACHE["nc"] = nc
    return nc


def _build_tail(nc, red, dot, pn2, tn2, d_cos):
    if True:
        if True:
            # cosine tail on [128, 2]
            pn = red.tile([128, 2], F32, tag="pn")
            tn = red.tile([128, 2], F32, tag="tn")
            rp = red.tile([128, 2], F32, tag="rp")
            rt = red.tile([128, 2], F32, tag="rt")
            den = red.tile([128, 2], F32, tag="den")
            cosv = red.tile([128, 2], F32, tag="cosv")
            nc.scalar.activation(pn[:], pn2[:], mybir.ActivationFunctionType.Sqrt)
            nc.scalar.activation(tn[:], tn2[:], mybir.ActivationFunctionType.Sqrt)
            # rp = 1/(pn+eps), rt = 1/(tn+eps)
            nc.vector.tensor_scalar_add(rp[:], pn[:], EPS)
            nc.vector.reciprocal(rp[:], rp[:])
            nc.vector.tensor_scalar_add(rt[:], tn[:], EPS)
            nc.vector.reciprocal(rt[:], rt[:])
            # dot_normalized = dot * rp * rt
            nc.vector.scalar_tensor_tensor(
                dot[:], dot[:], 0.0, rp[:], ALU.bypass, ALU.mult
            )
            nc.vector.scalar_tensor_tensor(
                dot[:], dot[:], 0.0, rt[:], ALU.bypass, ALU.mult
            )
            # pn_norm = clamp(pn * rp, eps); tn_norm likewise
            nc.vector.scalar_tensor_tensor(
                pn[:], pn[:], 0.0, rp[:], ALU.bypass, ALU.mult
            )
            nc.vector.scalar_tensor_tensor(
                tn[:], tn[:], 0.0, rt[:], ALU.bypass, ALU.mult
            )
            nc.vector.tensor_scalar_max(pn[:], pn[:], EPS)
            nc.vector.tensor_scalar_max(tn[:], tn[:], EPS)
            nc.vector.scalar_tensor_tensor(
                den[:], pn[:], 0.0, tn[:], ALU.bypass, ALU.mult
            )
            nc.vector.reciprocal(den[:], den[:])
            nc.vector.scalar_tensor_tensor(
                cosv[:], dot[:], 0.0, den[:], ALU.bypass, ALU.mult
            )
            nc.sync.dma_start(d_cos[:], cosv[:])


def kernel(pred_mz, pred_intensity, target_mz, target_intensity, target_mask):
    pred_mz = np.ascontiguousarray(pred_mz, dtype=np.float32)
    pred_intensity = np.ascontiguousarray(pred_intensity, dtype=np.float32)
    target_mz = np.ascontiguousarray(target_mz, dtype=np.float32)
    target_intensity = np.ascontiguousarray(target_intensity, dtype=np.float32)
    target_mask = np.ascontiguousarray(target_mask, dtype=np.float32)

    nc = build_nc()
    in_maps = []
    for c in range(N_CORES):
        rs = slice(c * ROWS_PER_CORE, (c + 1) * ROWS_PER_CORE)
        in_maps.append(
            {
                "pmz": pred_mz[rs],
                "pint": pred_intensity[rs],
                "tmz": target_mz[rs],
                "tint": target_intensity[rs],
                "tmask": target_mask[rs],
            }
        )
    res = run_bass_kernel_spmd(nc, in_maps, core_ids=list(range(N_CORES)))
    cos = np.concatenate(
        [r["cos"].T.reshape(-1) for r in res.results]
    )  # [2048] rows in order: core-major, tile, partition
    mean = np.mean(cos.astype(np.float64))
    return np.float32(1.0 - mean)


# revision 11
# speedup vs baseline: 24.8937x; 24.8937x over previous
"""CosineSimilarityLoss (histogram binning) Trainium2 kernel.

Full inputs [2048, 4096] f32 x5 -> scalar f32 loss = 1 - mean(cosine_sim).

Strategy: data-parallel over 8 cores (256 rows each). Per core, per
128-row tile: stream y = mz*2000 and intensity through a fused custom
DVE instruction  accum[p] = sum_k I[p,k] * [j <= y[p,k] < j+1]  --
one instruction per histogram bin -- building per-row histograms
PH/TH [128, 2000] exactly (bit-exact trunc semantics vs the reference's
astype(int32)). Dot products and squared norms via tensor_tensor_reduce,
cosine tail math on-chip; final mean on host.
"""

import sys

sys.path.insert(0, "/opt/trn_rl_repo")

from operator import add

import numpy as np

import concourse.bass as bass
from concourse import bacc, mybir
from concourse.bass_utils import run_bass_kernel_spmd
from concourse.tile import TileContext
from concourse.dve_ops import (
    DveOp,
    OPS,
    CUSTOM_DVE_SPECS,
    _SUB_OPCODE_FOR_NAME,
    _CUSTOM_DVE_ROW_BASE,
)
from concourse.dve_spec import Spec, Src0, Src1, C0, C1, Zero, lower
from concourse.dve_uop import DveOpSpec

import os

N_CORES = 8
B, P, T = 2048, 4096, 4096
ROWS_PER_CORE = B // N_CORES  # 256
NUM_BINS = int(os.environ.get("KERNEL_NUM_BINS", "2000"))
STAGE = os.environ.get("KERNEL_STAGE", "full")  # hist | red | full
EPS = 1e-8
F32 = mybir.dt.float32
ALU = mybir.AluOpType


def _register_op(name, spec):
    shas = {}
    for ver in ("v3", "v4"):
        s = DveOpSpec(name=name, opcode=0, uops=lower(spec, ver=ver), rd1_en=True)
        shas[ver] = s.sha(ver)
    op = DveOp(name, spec, subdim=False, uops_sha=shas)
    if name not in _SUB_OPCODE_FOR_NAME:
        OPS.append(op)
        CUSTOM_DVE_SPECS[name] = spec
        _SUB_OPCODE_FOR_NAME[name] = _CUSTOM_DVE_ROW_BASE + len(OPS) - 1
        assert _SUB_OPCODE_FOR_NAME[name] < 0x20
    return op


def _win_ref(in0, in1, s0, s1, imm2):
    b = ((in0 >= s0) & (in0 < s1)).astype(np.float32) * in1
    return b, b.reshape(b.shape[0], -1).sum(axis=-1, keepdims=True).astype(np.float32)


# accum_out[p] = sum_k in1[p,k] * [s0 <= in0[p,k] < s1]
WINMULRED = _register_op(
    "WINMULRED",
    Spec(
        body=(Src0 >= C0) * (Src0 < C1) * Src1,
        accum=add,
        accum_init=Zero,
        reference=_win_ref,
    ),
)

_NC_CACHE = {}


def build_nc():
    if "nc" in _NC_CACHE:
        return _NC_CACHE["nc"]
    nc = bacc.Bacc("TRN2", target_bir_lowering=False, debug=False, num_devices=N_CORES)
    d_pmz = nc.dram_tensor("pmz", [ROWS_PER_CORE, P], F32, kind="ExternalInput")
    d_pint = nc.dram_tensor("pint", [ROWS_PER_CORE, P], F32, kind="ExternalInput")
    d_tmz = nc.dram_tensor("tmz", [ROWS_PER_CORE, T], F32, kind="ExternalInput")
    d_tint = nc.dram_tensor("tint", [ROWS_PER_CORE, T], F32, kind="ExternalInput")
    d_tmask = nc.dram_tensor("tmask", [ROWS_PER_CORE, T], F32, kind="ExternalInput")
    d_cos = nc.dram_tensor("cos", [128, 2], F32, kind="ExternalOutput")

    n_tiles = ROWS_PER_CORE // 128  # 2

    with TileContext(nc) as tc:
        with (
            tc.tile_pool(name="io", bufs=1) as io,
            tc.tile_pool(name="hist", bufs=1) as hist,
            tc.tile_pool(name="red", bufs=1) as red,
        ):
            scr = red.tile([128, P], F32, tag="scr")
            dot = red.tile([128, 2], F32, tag="dot")
            pn2 = red.tile([128, 2], F32, tag="pn2")
            tn2 = red.tile([128, 2], F32, tag="tn2")

            for t in range(n_tiles):
                rs = slice(128 * t, 128 * (t + 1))
                yp = io.tile([128, P], F32, tag="yp")
                ip = io.tile([128, P], F32, tag="ip")
                yt = io.tile([128, T], F32, tag="yt")
                it = io.tile([128, T], F32, tag="it")
                tm = io.tile([128, T], F32, tag="tm")
                nc.sync.dma_start(yp[:], d_pmz[rs, :])
                nc.sync.dma_start(ip[:], d_pint[rs, :])
                nc.sync.dma_start(yt[:], d_tmz[rs, :])
                nc.sync.dma_start(it[:], d_tint[rs, :])
                nc.sync.dma_start(tm[:], d_tmask[rs, :])
                # y = mz * 2000  (single-rounded f32 mul, matches reference)
                nc.vector.tensor_scalar_mul(yp[:], yp[:], 2000.0)
                nc.vector.tensor_scalar_mul(yt[:], yt[:], 2000.0)
                # masked target intensity
                nc.vector.scalar_tensor_tensor(
                    it[:], it[:], 0.0, tm[:], ALU.bypass, ALU.mult
                )

                ph = hist.tile([128, NUM_BINS], F32, tag="ph")
                th = hist.tile([128, NUM_BINS], F32, tag="th")
                for j in range(NUM_BINS):
                    nc.vector._custom_dve(
                        WINMULRED,
                        out=scr[:],
                        accum_out=ph[:, j : j + 1],
                        in0=yp[:],
                        in1=ip[:],
                        s0=float(j),
                        s1=float(j + 1),
                    )
                for j in range(NUM_BINS):
                    nc.vector._custom_dve(
                        WINMULRED,
                        out=scr[:],
                        accum_out=th[:, j : j + 1],
                        in0=yt[:],
                        in1=it[:],
                        s0=float(j),
                        s1=float(j + 1),
                    )
                if STAGE == "hist":
                    continue
                hsc = red.tile([128, NUM_BINS], F32, tag="hsc")
                nc.vector.scalar_tensor_tensor(
                    hsc[:], ph[:], 0.0, th[:], ALU.bypass, ALU.mult,
                    accum_out=dot[:, t : t + 1],
                )
                nc.scalar.activation(
                    hsc[:], ph[:], mybir.ActivationFunctionType.Square,
                    accum_out=pn2[:, t : t + 1],
                )
                nc.scalar.activation(
                    hsc[:], th[:], mybir.ActivationFunctionType.Square,
                    accum_out=tn2[:, t : t + 1],
                )

            if STAGE == "hist":
                cz = red.tile([128, 2], F32, tag="cz")
                nc.vector.tensor_scalar_mul(cz[:], scr[:, 0:2], 0.0)
                nc.sync.dma_start(d_cos[:], cz[:])
            elif STAGE == "red":
                nc.sync.dma_start(d_cos[:], dot[:])
            else:
                _build_tail(nc, red, dot, pn2, tn2, d_cos)
    nc.compile()
    _NC_CACHE["nc"] = nc
    return nc


def _build_tail(nc, red, dot, pn2, tn2, d_cos):
    if True:
        if True:
            # cosine tail on [128, 2]
            pn = red.tile([128, 2], F32, tag="pn")
            tn = red.tile([128, 2], F32, tag="tn")
            rp = red.tile([128, 2], F32, tag="rp")
            rt = red.tile([128, 2], F32, tag="rt")
            den = red.tile([128, 2], F32, tag="den")
            cosv = red.tile([128, 2], F32, tag="cosv")
            nc.scalar.activation(pn[:], pn2[:], mybir.ActivationFunctionType.Sqrt)
            nc.scalar.activation(tn[:], tn2[:], mybir.ActivationFunctionType.Sqrt)
            # rp = 1/(pn+eps), rt = 1/(tn+eps)
            nc.vector.tensor_scalar_add(rp[:], pn[:], EPS)
            nc.vector.reciprocal(rp[:], rp[:])
            nc.vector.tensor_scalar_add(rt[:], tn[:], EPS)
            nc.vector.reciprocal(rt[:], rt[:])
            # dot_normalized = dot * rp * rt
            nc.vector.scalar_tensor_tensor(
                dot[:], dot[:], 0.0, rp[:], ALU.bypass, ALU.mult
            )
            nc.vector.scalar_tensor_tensor(
                dot[:], dot[:], 0.0, rt[:], ALU.bypass, ALU.mult
            )
            # pn_norm = clamp(pn * rp, eps); tn_norm likewise
            nc.vector.scalar_tensor_tensor(
                pn[:], pn[:], 0.0, rp[:], ALU.bypass, ALU.mult
            )
            nc.vector.scalar_tensor_tensor(
                tn[:], tn[:], 0.0, rt[:], ALU.bypass, ALU.mult
            )
            nc.vector.tensor_scalar_max(pn[:], pn[:], EPS)
            nc.vector.tensor_scalar_max(tn[:], tn[:], EPS)
            nc.vector.scalar_tensor_tensor(
                den[:], pn[:], 0.0, tn[:], ALU.bypass, ALU.mult
            )
            nc.vector.reciprocal(den[:], den[:])
            nc.vector.scalar_tensor_tensor(
                cosv[:], dot[:], 0.0, den[:], ALU.bypass, ALU.mult
            )
            nc.sync.dma_start(d_cos[:], cosv[:])


def make_in_maps(np_inputs):
    in_maps = []
    for c in range(N_CORES):
        rs = slice(c * ROWS_PER_CORE, (c + 1) * ROWS_PER_CORE)
        in_maps.append(
            {
                "pmz": np.ascontiguousarray(np_inputs["pred_mz"][rs]),
                "pint": np.ascontiguousarray(np_inputs["pred_intensity"][rs]),
                "tmz": np.ascontiguousarray(np_inputs["target_mz"][rs]),
                "tint": np.ascontiguousarray(np_inputs["target_intensity"][rs]),
                "tmask": np.ascontiguousarray(np_inputs["target_mask"][rs]),
            }
        )
    return in_maps


def kernel(pred_mz, pred_intensity, target_mz, target_intensity, target_mask):
    pred_mz = np.ascontiguousarray(pred_mz, dtype=np.float32)
    pred_intensity = np.ascontiguousarray(pred_intensity, dtype=np.float32)
    target_mz = np.ascontiguousarray(target_mz, dtype=np.float32)
    target_intensity = np.ascontiguousarray(target_intensity, dtype=np.float32)
    target_mask = np.ascontiguousarray(target_mask, dtype=np.float32)

    nc = build_nc()
    in_maps = make_in_maps(
        {
            "pred_mz": pred_mz,
            "pred_intensity": pred_intensity,
            "target_mz": target_mz,
            "target_intensity": target_intensity,
            "target_mask": target_mask,
        }
    )
    res = run_bass_kernel_spmd(nc, in_maps, core_ids=list(range(N_CORES)))
    cos = np.concatenate(
        [r["cos"].T.reshape(-1) for r in res.results]
    )  # [2048] rows in order: core-major, tile, partition
    mean = np.mean(cos.astype(np.float64))
    return np.float32(1.0 - mean)


# revision 14
# speedup vs baseline: 25.0415x; 1.0059x over previous
"""CosineSimilarityLoss (histogram binning) Trainium2 kernel.

Full inputs [2048, 4096] f32 x5 -> scalar f32 loss = 1 - mean(cosine_sim).

Strategy: data-parallel over 8 cores (256 rows each). Per core, per
128-row tile: stream y = mz*2000 and intensity through a fused custom
DVE instruction  accum[p] = sum_k I[p,k] * [j <= y[p,k] < j+1]  --
one instruction per histogram bin -- building per-row histograms
PH/TH [128, 2000] exactly (bit-exact trunc semantics vs the reference's
astype(int32)). Dot products and squared norms via tensor_tensor_reduce,
cosine tail math on-chip; final mean on host.
"""

import sys

sys.path.insert(0, "/opt/trn_rl_repo")

from operator import add

import numpy as np

import concourse.bass as bass
from concourse import bacc, mybir
from concourse.bass_utils import run_bass_kernel_spmd
from concourse.tile import TileContext
from concourse.dve_ops import (
    DveOp,
    OPS,
    CUSTOM_DVE_SPECS,
    _SUB_OPCODE_FOR_NAME,
    _CUSTOM_DVE_ROW_BASE,
)
from concourse.dve_spec import Spec, Src0, Src1, C0, C1, Zero, lower
from concourse.dve_uop import DveOpSpec

import os

N_CORES = 8
B, P, T = 2048, 4096, 4096
ROWS_PER_CORE = B // N_CORES  # 256
NUM_BINS = int(os.environ.get("KERNEL_NUM_BINS", "2000"))
STAGE = os.environ.get("KERNEL_STAGE", "full")  # hist | red | full
ITEMS_PER_OP = int(os.environ.get("KERNEL_ITEMS_PER_OP", "4096"))  # timing diag
HIST_MODE = os.environ.get("KERNEL_HIST_MODE", "win")  # win | stt
EPS = 1e-8
F32 = mybir.dt.float32
ALU = mybir.AluOpType


def _register_op(name, spec):
    shas = {}
    for ver in ("v3", "v4"):
        s = DveOpSpec(name=name, opcode=0, uops=lower(spec, ver=ver), rd1_en=True)
        shas[ver] = s.sha(ver)
    op = DveOp(name, spec, subdim=False, uops_sha=shas)
    if name not in _SUB_OPCODE_FOR_NAME:
        OPS.append(op)
        CUSTOM_DVE_SPECS[name] = spec
        _SUB_OPCODE_FOR_NAME[name] = _CUSTOM_DVE_ROW_BASE + len(OPS) - 1
        assert _SUB_OPCODE_FOR_NAME[name] < 0x20
    return op


def _win_ref(in0, in1, s0, s1, imm2):
    b = ((in0 >= s0) & (in0 < s1)).astype(np.float32) * in1
    return b, b.reshape(b.shape[0], -1).sum(axis=-1, keepdims=True).astype(np.float32)


# accum_out[p] = sum_k in1[p,k] * [s0 <= in0[p,k] < s1]
WINMULRED = _register_op(
    "WINMULRED",
    Spec(
        body=(Src0 >= C0) * (Src0 < C1) * Src1,
        accum=add,
        accum_init=Zero,
        reference=_win_ref,
    ),
)

_NC_CACHE = {}


def build_nc():
    if "nc" in _NC_CACHE:
        return _NC_CACHE["nc"]
    nc = bacc.Bacc("TRN2", target_bir_lowering=False, debug=False, num_devices=N_CORES)
    d_pmz = nc.dram_tensor("pmz", [ROWS_PER_CORE, P], F32, kind="ExternalInput")
    d_pint = nc.dram_tensor("pint", [ROWS_PER_CORE, P], F32, kind="ExternalInput")
    d_tmz = nc.dram_tensor("tmz", [ROWS_PER_CORE, T], F32, kind="ExternalInput")
    d_tint = nc.dram_tensor("tint", [ROWS_PER_CORE, T], F32, kind="ExternalInput")
    d_tmask = nc.dram_tensor("tmask", [ROWS_PER_CORE, T], F32, kind="ExternalInput")
    d_cos = nc.dram_tensor("cos", [128, 2], F32, kind="ExternalOutput")

    n_tiles = ROWS_PER_CORE // 128  # 2

    with TileContext(nc) as tc:
        with (
            tc.tile_pool(name="io", bufs=1) as io,
            tc.tile_pool(name="hist", bufs=1) as hist,
            tc.tile_pool(name="red", bufs=1) as red,
        ):
            scr = red.tile([128, P], F32, tag="scr")
            dot = red.tile([128, 2], F32, tag="dot")
            pn2 = red.tile([128, 2], F32, tag="pn2")
            tn2 = red.tile([128, 2], F32, tag="tn2")

            for t in range(n_tiles):
                rs = slice(128 * t, 128 * (t + 1))
                yp = io.tile([128, P], F32, tag="yp")
                ip = io.tile([128, P], F32, tag="ip")
                yt = io.tile([128, T], F32, tag="yt")
                it = io.tile([128, T], F32, tag="it")
                tm = io.tile([128, T], F32, tag="tm")
                nc.sync.dma_start(yp[:], d_pmz[rs, :])
                nc.sync.dma_start(ip[:], d_pint[rs, :])
                nc.sync.dma_start(yt[:], d_tmz[rs, :])
                nc.sync.dma_start(it[:], d_tint[rs, :])
                nc.sync.dma_start(tm[:], d_tmask[rs, :])
                # y = mz * 2000  (single-rounded f32 mul, matches reference)
                nc.vector.tensor_scalar_mul(yp[:], yp[:], 2000.0)
                nc.vector.tensor_scalar_mul(yt[:], yt[:], 2000.0)
                # masked target intensity
                nc.vector.scalar_tensor_tensor(
                    it[:], it[:], 0.0, tm[:], ALU.bypass, ALU.mult
                )

                ph = hist.tile([128, NUM_BINS], F32, tag="ph")
                th = hist.tile([128, NUM_BINS], F32, tag="th")
                W = ITEMS_PER_OP
                if HIST_MODE == "stt":
                    phc = hist.tile([128, NUM_BINS + 1], F32, tag="phc")
                    thc = hist.tile([128, NUM_BINS + 1], F32, tag="thc")
                    for cum, ys, ws in ((phc, yp, ip), (thc, yt, it)):
                        for j in range(NUM_BINS + 1):
                            nc.vector.scalar_tensor_tensor(
                                scr[:, :W], ys[:, :W], float(j), ws[:, :W],
                                ALU.is_ge, ALU.mult,
                                accum_out=cum[:, j : j + 1],
                            )
                    # bins = cum[j] - cum[j+1]
                    nc.vector.scalar_tensor_tensor(
                        ph[:], phc[:, 0:NUM_BINS], 0.0, phc[:, 1 : NUM_BINS + 1],
                        ALU.bypass, ALU.subtract,
                    )
                    nc.vector.scalar_tensor_tensor(
                        th[:], thc[:, 0:NUM_BINS], 0.0, thc[:, 1 : NUM_BINS + 1],
                        ALU.bypass, ALU.subtract,
                    )
                for j in ([] if HIST_MODE == "stt" else range(NUM_BINS)):
                    nc.vector._custom_dve(
                        WINMULRED,
                        out=scr[:, :W],
                        accum_out=ph[:, j : j + 1],
                        in0=yp[:, :W],
                        in1=ip[:, :W],
                        s0=float(j),
                        s1=float(j + 1),
                    )
                for j in ([] if HIST_MODE == "stt" else range(NUM_BINS)):
                    nc.vector._custom_dve(
                        WINMULRED,
                        out=scr[:, :W],
                        accum_out=th[:, j : j + 1],
                        in0=yt[:, :W],
                        in1=it[:, :W],
                        s0=float(j),
                        s1=float(j + 1),
                    )
                if STAGE == "hist":
                    continue
                hsc = red.tile([128, NUM_BINS], F32, tag="hsc")
                nc.vector.scalar_tensor_tensor(
                    hsc[:], ph[:], 0.0, th[:], ALU.bypass, ALU.mult,
                    accum_out=dot[:, t : t + 1],
                )
                nc.scalar.activation(
                    hsc[:], ph[:], mybir.ActivationFunctionType.Square,
                    accum_out=pn2[:, t : t + 1],
                )
                nc.scalar.activation(
                    hsc[:], th[:], mybir.ActivationFunctionType.Square,
                    accum_out=tn2[:, t : t + 1],
                )

            if STAGE == "hist":
                cz = red.tile([128, 2], F32, tag="cz")
                nc.vector.tensor_scalar_mul(cz[:], scr[:, 0:2], 0.0)
                nc.sync.dma_start(d_cos[:], cz[:])
            elif STAGE == "red":
                nc.sync.dma_start(d_cos[:], dot[:])
            else:
                _build_tail(nc, red, dot, pn2, tn2, d_cos)
    nc.compile()
    _NC_CACHE["nc"] = nc
    return nc


def _build_tail(nc, red, dot, pn2, tn2, d_cos):
    if True:
        if True:
            # cosine tail on [128, 2]
            pn = red.tile([128, 2], F32, tag="pn")
            tn = red.tile([128, 2], F32, tag="tn")
            rp = red.tile([128, 2], F32, tag="rp")
            rt = red.tile([128, 2], F32, tag="rt")
            den = red.tile([128, 2], F32, tag="den")
            cosv = red.tile([128, 2], F32, tag="cosv")
            nc.scalar.activation(pn[:], pn2[:], mybir.ActivationFunctionType.Sqrt)
            nc.scalar.activation(tn[:], tn2[:], mybir.ActivationFunctionType.Sqrt)
            # rp = 1/(pn+eps), rt = 1/(tn+eps)
            nc.vector.tensor_scalar_add(rp[:], pn[:], EPS)
            nc.vector.reciprocal(rp[:], rp[:])
            nc.vector.tensor_scalar_add(rt[:], tn[:], EPS)
            nc.vector.reciprocal(rt[:], rt[:])
            # dot_normalized = dot * rp * rt
            nc.vector.scalar_tensor_tensor(
                dot[:], dot[:], 0.0, rp[:], ALU.bypass, ALU.mult
            )
            nc.vector.scalar_tensor_tensor(
                dot[:], dot[:], 0.0, rt[:], ALU.bypass, ALU.mult
            )
            # pn_norm = clamp(pn * rp, eps); tn_norm likewise
            nc.vector.scalar_tensor_tensor(
                pn[:], pn[:], 0.0, rp[:], ALU.bypass, ALU.mult
            )
            nc.vector.scalar_tensor_tensor(
                tn[:], tn[:], 0.0, rt[:], ALU.bypass, ALU.mult
            )
            nc.vector.tensor_scalar_max(pn[:], pn[:], EPS)
            nc.vector.tensor_scalar_max(tn[:], tn[:], EPS)
            nc.vector.scalar_tensor_tensor(
                den[:], pn[:], 0.0, tn[:], ALU.bypass, ALU.mult
            )
            nc.vector.reciprocal(den[:], den[:])
            nc.vector.scalar_tensor_tensor(
                cosv[:], dot[:], 0.0, den[:], ALU.bypass, ALU.mult
            )
            nc.sync.dma_start(d_cos[:], cosv[:])


def make_in_maps(np_inputs):
    in_maps = []
    for c in range(N_CORES):
        rs = slice(c * ROWS_PER_CORE, (c + 1) * ROWS_PER_CORE)
        in_maps.append(
            {
                "pmz": np.ascontiguousarray(np_inputs["pred_mz"][rs]),
                "pint": np.ascontiguousarray(np_inputs["pred_intensity"][rs]),
                "tmz": np.ascontiguousarray(np_inputs["target_mz"][rs]),
                "tint": np.ascontiguousarray(np_inputs["target_intensity"][rs]),
                "tmask": np.ascontiguousarray(np_inputs["target_mask"][rs]),
            }
        )
    return in_maps


def kernel(pred_mz, pred_intensity, target_mz, target_intensity, target_mask):
    pred_mz = np.ascontiguousarray(pred_mz, dtype=np.float32)
    pred_intensity = np.ascontiguousarray(pred_intensity, dtype=np.float32)
    target_mz = np.ascontiguousarray(target_mz, dtype=np.float32)
    target_intensity = np.ascontiguousarray(target_intensity, dtype=np.float32)
    target_mask = np.ascontiguousarray(target_mask, dtype=np.float32)

    nc = build_nc()
    in_maps = make_in_maps(
        {
            "pred_mz": pred_mz,
            "pred_intensity": pred_intensity,
            "target_mz": target_mz,
            "target_intensity": target_intensity,
            "target_mask": target_mask,
        }
    )
    res = run_bass_kernel_spmd(nc, in_maps, core_ids=list(range(N_CORES)))
    cos = np.concatenate(
        [r["cos"].T.reshape(-1) for r in res.results]
    )  # [2048] rows in order: core-major, tile, partition
    mean = np.mean(cos.astype(np.float64))
    return np.float32(1.0 - mean)
